# revision 22
# baseline (speedup 1.0000x reference)
"""Tensor-parallel GQA attention kernel for 8 Trainium2 NeuronCores.

Sharding: head-parallel. Core c computes q heads [4c, 4c+4) and kv head c
(GQA group); the output projection is column-sharded after AllGathers of the
per-core context. The context AllGather is split into 4 seq quarters, each
triggered as soon as its q-tile's attention finishes, so the collectives
overlap attention and the output projection. Host concatenates the 8 output
shards.

Storage dtype is bf16 (fp16 matmuls run at half PE rate on TRN2 hardware;
bf16 runs at full rate and halves HBM traffic vs fp32). All matmul
accumulation is fp32 in PSUM; softmax denominators stay fp32.
"""

import math
import sys

import ml_dtypes
import numpy as np

sys.path.insert(0, "/opt/trn_rl_repo")

# ---- problem constants (hardcoded per harness contract) ----
DIM = 4096
N_HEADS = 32
N_KV_HEADS = 8
HEAD_DIM = 128
N_REP = 4
SEQ = 2048
BATCH = 1
NCORES = 8

P = 128
KO = DIM // P        # 32 contraction chunks
SQ = 512             # seq tile width (matmul moving free dim)
NSQ = SEQ // SQ      # 4
NKS = SEQ // P       # 16 key tiles of 128
NH_LOC = N_HEADS // NCORES   # 4 local q heads
MQKV = NH_LOC * HEAD_DIM + 2 * HEAD_DIM  # 768 rows of fused qkv projection
DOUT = DIM // NCORES  # 512 output columns per core
SCALE = 1.0 / math.sqrt(HEAD_DIM)

XB = 4               # k-chunks per xT load

_CACHE = {}


def _build():
    """Build and compile the Bass kernel once per process."""
    if "nc" in _CACHE:
        return _CACHE["nc"]

    import concourse.bacc as bacc
    import concourse.mybir as mybir
    import concourse.tile as tile
    from contextlib import ExitStack

    F32 = mybir.dt.float32
    F32R = mybir.dt.float32r
    BF16 = mybir.dt.bfloat16
    MULT = mybir.AluOpType.mult
    ADD = mybir.AluOpType.add
    SUB = mybir.AluOpType.subtract
    EXP = mybir.ActivationFunctionType.Exp
    LN = mybir.ActivationFunctionType.Ln

    nc = bacc.Bacc(None, target_bir_lowering=False, debug=False)

    xT = nc.declare_dram_parameter("xt", [P, NSQ, KO, SQ], BF16, isOutput=False)
    wqkv = nc.declare_dram_parameter("wqkv", [P, KO, MQKV], BF16, isOutput=False)
    wo = nc.declare_dram_parameter("wo", [P, KO, DOUT], BF16, isOutput=False)
    cosd = nc.declare_dram_parameter("cost", [P, SEQ], F32, isOutput=False)
    sind = nc.declare_dram_parameter("sint", [P, SEQ], F32, isOutput=False)
    maskd = nc.declare_dram_parameter("masks", [P, 4, SQ], BF16, isOutput=False)
    out = nc.declare_dram_parameter("o", [DOUT, SEQ], F32, isOutput=True)

    with tile.TileContext(nc) as tc, ExitStack() as stack, \
         nc.allow_low_precision(
             reason="bf16 storage; all matmul accumulation stays fp32 in PSUM"):
        singles = stack.enter_context(tc.tile_pool(name="singles", bufs=1))
        dram = stack.enter_context(tc.tile_pool(name="dram", bufs=1, space="DRAM"))

        # one AllGather per seq quarter, fired as each q-tile finishes
        cc_in = [dram.tile([NH_LOC * HEAD_DIM, SQ], BF16, name=f"ccin{j}")
                 for j in range(NSQ)]
        cc_out = [dram.tile([N_HEADS * HEAD_DIM, SQ], BF16, addr_space="Shared",
                            name=f"ccout{j}") for j in range(NSQ)]

        # small constants via vector engine (cheap, no DMA)
        ones_f = singles.tile([P, 1], F32)
        nc.vector.memset(ones_f[:], 1.0)
        ones_col = singles.tile([P, 1], BF16)
        nc.vector.tensor_copy(ones_col[:], ones_f[:])
        ones_row_f = singles.tile([1, P], F32)
        nc.vector.memset(ones_row_f[:], 1.0)
        ones_row = singles.tile([1, P], F32R)
        nc.vector.tensor_copy(ones_row[:], ones_row_f[:])
        negb = singles.tile([P, 1], F32)
        nc.vector.memset(negb[:], -5.0)

        # attention operands, resident across phases 1-2. Per-sq tiles so
        # phase-2 reads only depend on the sq quarter that produced them
        # (a single tile would serialize phase 2 behind ALL of phase 1).
        qsb = [singles.tile([P, NH_LOC, SQ], BF16, name=f"qsb{s}")
               for s in range(NSQ)]
        kTsb = [singles.tile([P, SQ], BF16, name=f"kt{s}") for s in range(NSQ)]
        vTsb = [singles.tile([P, SQ], BF16, name=f"vt{s}") for s in range(NSQ)]
        vsb = [singles.tile([P, 4, HEAD_DIM], BF16, name=f"v{s}")
               for s in range(NSQ)]

        cos_sb = singles.tile([P, SEQ], F32)
        sin_sb = singles.tile([P, SEQ], F32)
        mask_sb = singles.tile([P, 4, SQ], BF16)
        wo_sb = singles.tile([P, KO, DOUT], BF16)

        # ---------------- Phase 1: fused QKV projection + RoPE ----------------
        # m-tile order chosen so PSUM tiles are revisited in the order the
        # RoPE eviction frees them (pairs (0,3), (1,4), (2,5)).
        M_ORDER = (0, 3, 1, 4, 2, 5)
        M_ORDER_LAST = (2, 5, 0, 3, 1, 4)  # last k-chunk of last sq: stop k/v first
        with tc.tile_pool(name="wq", bufs=1) as wpool, \
             tc.tile_pool(name="xtp", bufs=4) as xpool, \
             tc.tile_pool(name="rt", bufs=2) as rpool, \
             tc.tile_pool(name="cp", bufs=2) as cpool, \
             tc.tile_pool(name="ps1", bufs=1, space="PSUM") as pp1:
            w = [None] * (KO // 4)

            def load_wg(g):
                wg = wpool.tile([P, 4, MQKV], BF16, tag=f"w{g}", name=f"w{g}")
                nc.sync.dma_start(wg[:], wqkv[:, 4 * g:4 * g + 4, :])
                w[g] = wg

            XCHUNKS = [(sq, xb) for sq in range(NSQ) for xb in range(KO // XB)]
            xtiles = {}

            def load_xk(i):
                sq, xb = XCHUNKS[i]
                xk = xpool.tile([P, XB, SQ], BF16, tag="xt", name=f"x{sq}_{xb}")
                nc.sync.dma_start(xk[:], xT[:, sq, xb * XB:(xb + 1) * XB, :])
                xtiles[i] = xk

            # startup order: weight groups and x tiles interleaved to match
            # the consumption order (PE eats one wg + one xk per 24 matmuls)
            # — the 8 cores' initial HBM burst is bandwidth-bound, so arrival
            # order is everything. masks are tiny; cos/sin are needed at the
            # first RoPE (~60us in); wo only at phase 3.
            load_wg(0)
            load_xk(0)
            nc.scalar.dma_start(mask_sb[:], maskd[:])
            load_wg(1)
            load_xk(1)
            load_wg(2)
            load_xk(2)
            for g in range(3, KO // 4):
                load_wg(g)
            nc.scalar.dma_start(cos_sb[:], cosd[:])
            nc.scalar.dma_start(sin_sb[:], sind[:])

            def wslice(k, m):
                return w[k // 4][:, k % 4, m * P:(m + 1) * P]

            for sq in range(NSQ):
                cols = slice(sq * SQ, (sq + 1) * SQ)
                pq = [pp1.tile([P, SQ], F32, tag=f"p{m}", name=f"p{m}_{sq}")
                      for m in range(6)]
                for xb in range(KO // XB):
                    i = sq * (KO // XB) + xb
                    if i + 3 < len(XCHUNKS):
                        load_xk(i + 3)
                    xk = xtiles.pop(i)
                    for kk in range(XB):
                        k = xb * XB + kk
                        morder = (M_ORDER_LAST if (sq == NSQ - 1 and k == KO - 1)
                                  else M_ORDER)
                        for m in morder:
                            nc.tensor.matmul(pq[m][:], wslice(k, m), xk[:, kk, :],
                                             start=(k == 0), stop=(k == KO - 1))

                # Evict PSUM -> SBUF fp32 on the (idle) Scalar engine first:
                # banks free ~0.5us per tile instead of waiting for the whole
                # DVE RoPE chain, so the next sq's matmuls (and phase 2's PSUM
                # pool, which needs every bank) never stall on the vector
                # engine. RoPE then runs from SBUF off the critical path.
                corder = M_ORDER_LAST if sq == NSQ - 1 else M_ORDER
                pcp = [None] * 6
                for m in corder:
                    pcp[m] = cpool.tile([P, SQ], F32, tag=f"c{m}",
                                        name=f"c{m}_{sq}")
                    nc.scalar.copy(pcp[m][:], pq[m][:])

                # RoPE. m-tile pairs: (0,3)->(q0,q1), (1,4)->(q2,q3),
                # (2,5)->(k | v-halves). Last sq handles k/v first so
                # attention + the v transposes start without waiting for the
                # q combines.
                groups = (2, 0, 1) if sq == NSQ - 1 else (0, 1, 2)
                for i in groups:
                    A, B = pcp[i][:], pcp[i + 3][:]
                    tac = rpool.tile([P, SQ], BF16, tag="tac")   # A*cos
                    tas = rpool.tile([P, SQ], BF16, tag="tas")   # A*sin
                    tbs = rpool.tile([P, SQ], BF16, tag="tbs")   # B*sin
                    tbc = rpool.tile([P, SQ], BF16, tag="tbc")   # B*cos
                    nc.vector.tensor_tensor(tac[:], A, cos_sb[:, cols], MULT)
                    nc.vector.tensor_tensor(tas[:], A, sin_sb[:, cols], MULT)
                    if i == 2:
                        # v passthrough straight from PSUM (frees pq[2]/pq[5])
                        nc.vector.tensor_copy(vTsb[sq][0:64, :], A[64:128])
                    nc.vector.tensor_tensor(tbs[:], B, sin_sb[:, cols], MULT)
                    nc.vector.tensor_tensor(tbc[:], B, cos_sb[:, cols], MULT)
                    if i == 2:
                        nc.vector.tensor_copy(vTsb[sq][64:128, :], B[64:128])
                        dests = ((slice(0, 64), kTsb[sq][0:64, :],
                                  kTsb[sq][64:128, :]),)
                    else:
                        h0q, h1q = 2 * i, 2 * i + 1
                        dests = ((slice(0, 64), qsb[sq][0:64, h0q, :],
                                  qsb[sq][64:128, h0q, :]),
                                 (slice(64, 128), qsb[sq][0:64, h1q, :],
                                  qsb[sq][64:128, h1q, :]))
                    for half, dre, dim_ in dests:
                        nc.vector.tensor_tensor(dre, tac[half], tbs[half], SUB)
                        nc.vector.tensor_tensor(dim_, tas[half], tbc[half], ADD)
                    if i == 2:
                        # transpose this quarter's v: vT [128, s] -> v [s, 128]
                        # via DMA xbar transpose (off the PE queue, ACT ring)
                        for t in range(4):
                            nc.scalar.dma_start_transpose(
                                vsb[sq][:, t, :],
                                vTsb[sq][:, t * P:(t + 1) * P])

                if sq == 0:
                    # wo is needed only at phase 3; issue mid-phase-1 when the
                    # startup burst has drained
                    nc.scalar.dma_start(wo_sb[:], wo[:])

        # ---------- Phases 2+3: attention + output projection ----------
        # One pool context for both phases (closing a pool inserts an
        # all-engine barrier). Phase 2 runs as a single flat chunk stream
        # with a global 3-deep score->PV pipeline, so tile boundaries never
        # drain the ACT/DVE pipelines. Phase 3 reuses phase 2's PSUM tags.
        with tc.tile_pool(name="pt", bufs=6) as ptpool, \
             tc.tile_pool(name="st", bufs=4) as stpool, \
             tc.tile_pool(name="os", bufs=2) as ospool, \
             tc.tile_pool(name="ps2", bufs=1, space="PSUM") as pp2:

            pending_fin = [None]

            def emit_fin():
                if pending_fin[0] is not None:
                    fn, j, h = pending_fin[0]
                    pending_fin[0] = None
                    fn()
                    if h == NH_LOC - 1:
                        nc.gpsimd.collective_compute(
                            "AllGather", mybir.AluOpType.bypass,
                            replica_groups=[list(range(NCORES))],
                            ins=[cc_in[j][:]], outs=[cc_out[j][:]])

            class T2:
                __slots__ = ("j", "h", "nks", "ps_ctx", "ps_den")

                def __init__(self, j, h):
                    self.j, self.h, self.nks = j, h, 4 * (j + 1)
                    self.ps_ctx = self.ps_den = None

            def q_off(tl, t):
                return (t - 4 * tl.j) * P if t >= 4 * tl.j else 0

            def do_scores(tl, t):
                # Ragged diagonal: chunk t only attends queries >= its key
                # block, so compute just cols [q0:] and mask the one 128-col
                # triangle strip.
                j, h = tl.j, tl.h
                q0 = q_off(tl, t)
                ps_s = pp2.tile([P, SQ], F32, tag="s", bufs=3,
                                name=f"s{h}_{j}_{t}")
                nc.tensor.matmul(ps_s[:, q0:],
                                 kTsb[t // 4][:, (t % 4) * P:(t % 4 + 1) * P],
                                 qsb[j][:, h, q0:], start=True, stop=True)
                pT = ptpool.tile([P, SQ], BF16, tag="pT", name=f"pT{h}_{j}_{t}")
                # bias keeps exp small (cancels in softmax)
                nc.scalar.activation(pT[:, q0:], ps_s[:, q0:], EXP,
                                     scale=SCALE, bias=negb[:])
                if t >= 4 * j:
                    nc.vector.tensor_tensor(pT[:, q0:q0 + P], pT[:, q0:q0 + P],
                                            mask_sb[:, 0, 0:P], MULT)
                return pT

            def do_pv(tl, t, pT, first, last):
                if first:
                    # first processed chunk is always full-width, so every
                    # PSUM column is initialized by its start=True
                    tl.ps_ctx = pp2.tile([P, SQ], F32, tag="ctx", bufs=2,
                                         name=f"ctx{tl.h}_{tl.j}")
                    tl.ps_den = pp2.tile([1, SQ], F32, tag="den", bufs=2,
                                         name=f"den{tl.h}_{tl.j}")
                q0 = q_off(tl, t)
                nc.tensor.matmul(tl.ps_ctx[:, q0:], vsb[t // 4][:, t % 4, :],
                                 pT[:, q0:], start=first, stop=last)
                nc.tensor.matmul(tl.ps_den[0:1, q0:], ones_col[:], pT[:, q0:],
                                 start=first, stop=last)
                if last:
                    # 1/den via exp(-ln(den)) on ACT: Ln and Exp share one
                    # activation table set (no table switch with the attention
                    # exps), and the DVE's multi-pass reciprocal (3.4us per
                    # call) stays off the epilogue chain entirely.
                    lnden = stpool.tile([1, SQ], F32, tag="lnden")
                    nc.scalar.activation(lnden[:], tl.ps_den[0:1, :], LN)
                    rc_sb = stpool.tile([1, SQ], F32R, tag="rc_sb")
                    nc.scalar.activation(rc_sb[:], lnden[:], EXP, scale=-1.0)

                    def fin(tl=tl, rc_sb=rc_sb):
                        ps_bc = pp2.tile([P, SQ], F32, tag="bc", bufs=1,
                                         name=f"bc{tl.h}_{tl.j}")
                        nc.tensor.matmul(ps_bc[:], ones_row[:], rc_sb[:],
                                         start=True, stop=True)
                        bc_sb = stpool.tile([P, SQ], F32, tag="bc_sb")
                        nc.vector.tensor_copy(bc_sb[:], ps_bc[:])
                        ctx_st = stpool.tile([P, SQ], BF16, tag="ctx_st")
                        nc.vector.tensor_tensor(ctx_st[:], tl.ps_ctx[:],
                                                bc_sb[:], MULT)
                        nc.sync.dma_start(
                            cc_in[tl.j].rearrange("(h p) s -> p h s", p=P)
                                       [:, tl.h, :],
                            ctx_st[:])

                    pending_fin[0] = (fin, tl.j, tl.h)

            # flat stream over all (tile, chunk): diagonal chunks first
            # within each tile (tail flush then waits only on exp, not
            # exp+mask)
            stream = []
            for j in range(NSQ):
                for h in range(NH_LOC):
                    tl = T2(j, h)
                    order = list(range(4 * j, tl.nks)) + list(range(4 * j))
                    fin_t = 6 if tl.nks >= 8 else 3
                    for pos, t in enumerate(order):
                        stream.append((tl, t, pos, fin_t))

            from collections import deque
            pend = deque()
            for tl, t, pos, fin_t in stream:
                pend.append((tl, t, pos, do_scores(tl, t)))
                if pos == fin_t:
                    emit_fin()
                if len(pend) > 3:
                    ptl, pt, ppos, pT = pend.popleft()
                    do_pv(ptl, pt, pT, ppos == 0, ppos == ptl.nks - 1)
            while pend:
                ptl, pt, ppos, pT = pend.popleft()
                do_pv(ptl, pt, pT, ppos == 0, ppos == ptl.nks - 1)
            emit_fin()

            # ---- Phase 3: output projection (column shard) ----
            # cxt loads ride the ACT ring (the SP ring still has phase-2's
            # last cc_in writes queued); PSUM eviction goes to the idle
            # Vector engine. PSUM accumulators reuse phase-2 tags.
            for part in range(NSQ):
                ccv = cc_out[part].rearrange("(ko p) s -> p ko s", p=P)
                po = [pp2.tile([P, SQ], F32, tag="s", bufs=3,
                               name=f"o0_{part}"),
                      pp2.tile([P, SQ], F32, tag="s", bufs=3,
                               name=f"o1_{part}"),
                      pp2.tile([P, SQ], F32, tag="ctx", bufs=2,
                               name=f"o2_{part}"),
                      pp2.tile([P, SQ], F32, tag="ctx", bufs=2,
                               name=f"o3_{part}")]
                for ko in range(KO):
                    # cxt shares the pT pool: the slot-reuse dependency keeps
                    # the scheduler from hoisting these AG-gated loads ahead
                    # of phase-2's exps on the ACT ring (which would stall
                    # the exp stream behind a collective wait)
                    cxt = ptpool.tile([P, SQ], BF16, tag="pT",
                                      name=f"cx{part}_{ko}")
                    nc.scalar.dma_start(cxt[:], ccv[:, ko, :])
                    for dt in range(4):
                        nc.tensor.matmul(
                            po[dt][:],
                            wo_sb[:, ko, dt * P:(dt + 1) * P],
                            cxt[:],
                            start=(ko == 0), stop=(ko == KO - 1))
                out_sb = ospool.tile([P, 4, SQ], F32, tag="osb",
                                     name=f"osb{part}")
                for dt in range(4):
                    nc.vector.tensor_copy(out_sb[:, dt, :], po[dt][:])
                nc.sync.dma_start(
                    out.rearrange("(dt p) s -> p dt s", p=P)
                       [:, :, part * SQ:(part + 1) * SQ],
                    out_sb[:])

    nc.compile()
    _CACHE["nc"] = nc
    return nc


def _prep_inputs(x, wq, wk, wv, wo, freqs_cos, freqs_sin):
    """Host-side sharding + layout prep. Returns in_maps for the 8 cores."""
    x = np.asarray(x, dtype=np.float32)
    wq = np.asarray(wq, dtype=np.float32)
    wk = np.asarray(wk, dtype=np.float32)
    wv = np.asarray(wv, dtype=np.float32)
    wo = np.asarray(wo, dtype=np.float32)
    freqs_cos = np.asarray(freqs_cos, dtype=np.float32)
    freqs_sin = np.asarray(freqs_sin, dtype=np.float32)

    # xT in [P, NSQ, KO, SQ] layout: element (d, s), d = ko*128 + p, s = sq*SQ + s'
    xT = np.ascontiguousarray(
        x[0].T.reshape(KO, P, NSQ, SQ).transpose(1, 2, 0, 3)).astype(
            ml_dtypes.bfloat16)

    # rotate-half permutation within a head: [0,2,4,...126, 1,3,...,127]
    perm = np.concatenate([np.arange(0, HEAD_DIM, 2), np.arange(1, HEAD_DIM, 2)])

    # cos/sin tables transposed and duplicated across both 64-row halves
    cosT = np.ascontiguousarray(freqs_cos.T)  # [64, SEQ]
    sinT = np.ascontiguousarray(freqs_sin.T)
    cos2 = np.concatenate([cosT, cosT], axis=0)  # [128, SEQ]
    sin2 = np.concatenate([sinT, sinT], axis=0)

    # causal mask tiles: mask_r[i, jl] = 1 if jl - i >= 128*r
    i_idx = np.arange(P)[:, None]
    j_idx = np.arange(SQ)[None, :]
    masks = np.stack([(j_idx - i_idx >= P * r) for r in range(4)],
                     axis=0).astype(ml_dtypes.bfloat16)  # [4, 128, SQ]
    masks_l = np.ascontiguousarray(masks.transpose(1, 0, 2))  # [P, 4, SQ]

    in_maps = []
    for c in range(NCORES):
        # fused qkv weight rows, permuted for RoPE (re/im separated by m-tile)
        qh = [wq[(4 * c + h) * HEAD_DIM:(4 * c + h + 1) * HEAD_DIM][perm]
              for h in range(NH_LOC)]  # each [128, DIM], rows [re(64); im(64)]
        kh = wk[c * HEAD_DIM:(c + 1) * HEAD_DIM][perm]  # [128, DIM]
        vh = wv[c * HEAD_DIM:(c + 1) * HEAD_DIM]        # [128, DIM] original order
        rows = np.empty((MQKV, DIM), dtype=np.float32)
        rows[0:64] = qh[0][0:64]        # tile0: q0 re | q1 re
        rows[64:128] = qh[1][0:64]
        rows[128:192] = qh[2][0:64]     # tile1: q2 re | q3 re
        rows[192:256] = qh[3][0:64]
        rows[256:320] = kh[0:64]        # tile2: k re | v dims 0:64
        rows[320:384] = vh[0:64]
        rows[384:448] = qh[0][64:128]   # tile3: q0 im | q1 im
        rows[448:512] = qh[1][64:128]
        rows[512:576] = qh[2][64:128]   # tile4: q2 im | q3 im
        rows[576:640] = qh[3][64:128]
        rows[640:704] = kh[64:128]      # tile5: k im | v dims 64:128
        rows[704:768] = vh[64:128]
        wqkvT = np.ascontiguousarray(
            rows.T.reshape(KO, P, MQKV).transpose(1, 0, 2)).astype(
                ml_dtypes.bfloat16)

        woT = np.ascontiguousarray(
            wo[c * DOUT:(c + 1) * DOUT].T.reshape(KO, P, DOUT)
            .transpose(1, 0, 2)).astype(ml_dtypes.bfloat16)

        in_maps.append({
            "xt": xT,
            "wqkv": wqkvT,
            "wo": woT,
            "cost": cos2,
            "sint": sin2,
            "masks": masks_l,
        })
    return in_maps


def run(inputs, trace=False, tmpdir=None):
    """Compile (cached), run on 8 cores, return (output, BassKernelResults)."""
    from concourse.bass_utils import run_bass_kernel_spmd

    nc = _build()
    in_maps = _prep_inputs(**inputs)
    res = run_bass_kernel_spmd(nc, in_maps, list(range(NCORES)),
                               trace=trace, tmpdir=tmpdir)
    out = np.empty((BATCH, SEQ, DIM), dtype=np.float32)
    for c in range(NCORES):
        out[0, :, c * DOUT:(c + 1) * DOUT] = res.results[c]["o"].T
    return out, res


def kernel(**inputs) -> np.ndarray:
    out, _ = run(inputs)
    return out


# revision 23
# speedup vs baseline: 1.0307x; 1.0307x over previous
"""Tensor-parallel GQA attention kernel for 8 Trainium2 NeuronCores.

Sharding: head-parallel. Core c computes q heads [4c, 4c+4) and kv head c
(GQA group); the output projection is column-sharded after AllGathers of the
per-core context. The context AllGather is split into 4 seq quarters, each
triggered as soon as its q-tile's attention finishes, so the collectives
overlap attention and the output projection. Host concatenates the 8 output
shards.

Storage dtype is bf16 (fp16 matmuls run at half PE rate on TRN2 hardware;
bf16 runs at full rate and halves HBM traffic vs fp32). All matmul
accumulation is fp32 in PSUM; softmax denominators stay fp32.
"""

import math
import sys

import ml_dtypes
import numpy as np

sys.path.insert(0, "/opt/trn_rl_repo")

# ---- problem constants (hardcoded per harness contract) ----
DIM = 4096
N_HEADS = 32
N_KV_HEADS = 8
HEAD_DIM = 128
N_REP = 4
SEQ = 2048
BATCH = 1
NCORES = 8

P = 128
KO = DIM // P        # 32 contraction chunks
SQ = 512             # seq tile width (matmul moving free dim)
NSQ = SEQ // SQ      # 4
NKS = SEQ // P       # 16 key tiles of 128
NH_LOC = N_HEADS // NCORES   # 4 local q heads
MQKV = NH_LOC * HEAD_DIM + 2 * HEAD_DIM  # 768 rows of fused qkv projection
DOUT = DIM // NCORES  # 512 output columns per core
SCALE = 1.0 / math.sqrt(HEAD_DIM)

XB = 8               # k-chunks per xT load

_CACHE = {}


def _build():
    """Build and compile the Bass kernel once per process."""
    if "nc" in _CACHE:
        return _CACHE["nc"]

    import concourse.bacc as bacc
    import concourse.mybir as mybir
    import concourse.tile as tile
    from contextlib import ExitStack

    F32 = mybir.dt.float32
    F32R = mybir.dt.float32r
    BF16 = mybir.dt.bfloat16
    MULT = mybir.AluOpType.mult
    ADD = mybir.AluOpType.add
    SUB = mybir.AluOpType.subtract
    EXP = mybir.ActivationFunctionType.Exp
    LN = mybir.ActivationFunctionType.Ln

    nc = bacc.Bacc(None, target_bir_lowering=False, debug=False)

    xT = nc.declare_dram_parameter("xt", [P, NSQ, KO, SQ], BF16, isOutput=False)
    wqkv = nc.declare_dram_parameter("wqkv", [P, KO, MQKV], BF16, isOutput=False)
    wo = nc.declare_dram_parameter("wo", [P, KO, DOUT], BF16, isOutput=False)
    cosd = nc.declare_dram_parameter("cost", [P, SEQ], F32, isOutput=False)
    sind = nc.declare_dram_parameter("sint", [P, SEQ], F32, isOutput=False)
    maskd = nc.declare_dram_parameter("masks", [P, 4, SQ], BF16, isOutput=False)
    out = nc.declare_dram_parameter("o", [DOUT, SEQ], F32, isOutput=True)

    with tile.TileContext(nc) as tc, ExitStack() as stack, \
         nc.allow_low_precision(
             reason="bf16 storage; all matmul accumulation stays fp32 in PSUM"):
        singles = stack.enter_context(tc.tile_pool(name="singles", bufs=1))
        dram = stack.enter_context(tc.tile_pool(name="dram", bufs=1, space="DRAM"))

        # one AllGather per seq quarter, fired as each q-tile finishes
        cc_in = [dram.tile([NH_LOC * HEAD_DIM, SQ], BF16, name=f"ccin{j}")
                 for j in range(NSQ)]
        cc_out = [dram.tile([N_HEADS * HEAD_DIM, SQ], BF16, addr_space="Shared",
                            name=f"ccout{j}") for j in range(NSQ)]

        # small constants via vector engine (cheap, no DMA)
        ones_f = singles.tile([P, 1], F32)
        nc.vector.memset(ones_f[:], 1.0)
        ones_col = singles.tile([P, 1], BF16)
        nc.vector.tensor_copy(ones_col[:], ones_f[:])
        ones_row_f = singles.tile([1, P], F32)
        nc.vector.memset(ones_row_f[:], 1.0)
        ones_row = singles.tile([1, P], F32R)
        nc.vector.tensor_copy(ones_row[:], ones_row_f[:])
        negb = singles.tile([P, 1], F32)
        nc.vector.memset(negb[:], -5.0)

        # attention operands, resident across phases 1-2. Per-sq tiles so
        # phase-2 reads only depend on the sq quarter that produced them
        # (a single tile would serialize phase 2 behind ALL of phase 1).
        qsb = [singles.tile([P, NH_LOC, SQ], BF16, name=f"qsb{s}")
               for s in range(NSQ)]
        kTsb = [singles.tile([P, SQ], BF16, name=f"kt{s}") for s in range(NSQ)]
        vTsb = [singles.tile([P, SQ], BF16, name=f"vt{s}") for s in range(NSQ)]
        vsb = [singles.tile([P, 4, HEAD_DIM], BF16, name=f"v{s}")
               for s in range(NSQ)]

        cos_sb = singles.tile([P, SEQ], F32)
        sin_sb = singles.tile([P, SEQ], F32)
        mask_sb = singles.tile([P, 4, SQ], BF16)
        wo_sb = singles.tile([P, KO, DOUT], BF16)

        # ---------------- Phase 1: fused QKV projection + RoPE ----------------
        # m-tile order chosen so PSUM tiles are revisited in the order the
        # RoPE eviction frees them (pairs (0,3), (1,4), (2,5)).
        M_ORDER = (0, 3, 1, 4, 2, 5)
        M_ORDER_LAST = (2, 5, 0, 3, 1, 4)  # last k-chunk of last sq: stop k/v first
        with tc.tile_pool(name="wq", bufs=1) as wpool, \
             tc.tile_pool(name="xtp", bufs=3) as xpool, \
             tc.tile_pool(name="rt", bufs=2) as rpool, \
             tc.tile_pool(name="cp", bufs=2) as cpool, \
             tc.tile_pool(name="ps1", bufs=1, space="PSUM") as pp1:
            w = [None] * (KO // 4)

            def load_wg(g):
                wg = wpool.tile([P, 4, MQKV], BF16, tag=f"w{g}", name=f"w{g}")
                nc.sync.dma_start(wg[:], wqkv[:, 4 * g:4 * g + 4, :])
                w[g] = wg

            XCHUNKS = [(sq, xb) for sq in range(NSQ) for xb in range(KO // XB)]
            xtiles = {}

            def load_xk(i):
                sq, xb = XCHUNKS[i]
                xk = xpool.tile([P, XB, SQ], BF16, tag="xt", name=f"x{sq}_{xb}")
                nc.sync.dma_start(xk[:], xT[:, sq, xb * XB:(xb + 1) * XB, :])
                xtiles[i] = xk

            # startup order: weight groups and x tiles interleaved to match
            # the consumption order (PE eats one wg + one xk per 24 matmuls)
            # — the 8 cores' initial HBM burst is bandwidth-bound, so arrival
            # order is everything. masks are tiny; cos/sin are needed at the
            # first RoPE (~60us in); wo only at phase 3.
            load_wg(0)
            load_xk(0)
            nc.scalar.dma_start(mask_sb[:], maskd[:])
            load_wg(1)
            load_wg(2)
            load_xk(1)
            load_wg(3)
            for g in range(4, KO // 4):
                load_wg(g)
            nc.scalar.dma_start(cos_sb[:], cosd[:])
            nc.scalar.dma_start(sin_sb[:], sind[:])

            def wslice(k, m):
                return w[k // 4][:, k % 4, m * P:(m + 1) * P]

            for sq in range(NSQ):
                cols = slice(sq * SQ, (sq + 1) * SQ)
                pq = [pp1.tile([P, SQ], F32, tag=f"p{m}", name=f"p{m}_{sq}")
                      for m in range(6)]
                for xb in range(KO // XB):
                    i = sq * (KO // XB) + xb
                    if i + 2 < len(XCHUNKS):
                        load_xk(i + 2)
                    xk = xtiles.pop(i)
                    for kk in range(XB):
                        k = xb * XB + kk
                        morder = (M_ORDER_LAST if (sq == NSQ - 1 and k == KO - 1)
                                  else M_ORDER)
                        for m in morder:
                            nc.tensor.matmul(pq[m][:], wslice(k, m), xk[:, kk, :],
                                             start=(k == 0), stop=(k == KO - 1))

                # Evict PSUM -> SBUF fp32 on the (idle) Scalar engine first:
                # banks free ~0.5us per tile instead of waiting for the whole
                # DVE RoPE chain, so the next sq's matmuls (and phase 2's PSUM
                # pool, which needs every bank) never stall on the vector
                # engine. RoPE then runs from SBUF off the critical path.
                corder = M_ORDER_LAST if sq == NSQ - 1 else M_ORDER
                pcp = [None] * 6
                for m in corder:
                    pcp[m] = cpool.tile([P, SQ], F32, tag=f"c{m}",
                                        name=f"c{m}_{sq}")
                    nc.scalar.copy(pcp[m][:], pq[m][:])

                # RoPE. m-tile pairs: (0,3)->(q0,q1), (1,4)->(q2,q3),
                # (2,5)->(k | v-halves). Last sq handles k/v first so
                # attention + the v transposes start without waiting for the
                # q combines.
                groups = (2, 0, 1) if sq == NSQ - 1 else (0, 1, 2)
                for i in groups:
                    A, B = pcp[i][:], pcp[i + 3][:]
                    tac = rpool.tile([P, SQ], BF16, tag="tac")   # A*cos
                    tas = rpool.tile([P, SQ], BF16, tag="tas")   # A*sin
                    tbs = rpool.tile([P, SQ], BF16, tag="tbs")   # B*sin
                    tbc = rpool.tile([P, SQ], BF16, tag="tbc")   # B*cos
                    nc.vector.tensor_tensor(tac[:], A, cos_sb[:, cols], MULT)
                    nc.vector.tensor_tensor(tas[:], A, sin_sb[:, cols], MULT)
                    if i == 2:
                        # v passthrough straight from PSUM (frees pq[2]/pq[5])
                        nc.vector.tensor_copy(vTsb[sq][0:64, :], A[64:128])
                    nc.vector.tensor_tensor(tbs[:], B, sin_sb[:, cols], MULT)
                    nc.vector.tensor_tensor(tbc[:], B, cos_sb[:, cols], MULT)
                    if i == 2:
                        nc.vector.tensor_copy(vTsb[sq][64:128, :], B[64:128])
                        dests = ((slice(0, 64), kTsb[sq][0:64, :],
                                  kTsb[sq][64:128, :]),)
                    else:
                        h0q, h1q = 2 * i, 2 * i + 1
                        dests = ((slice(0, 64), qsb[sq][0:64, h0q, :],
                                  qsb[sq][64:128, h0q, :]),
                                 (slice(64, 128), qsb[sq][0:64, h1q, :],
                                  qsb[sq][64:128, h1q, :]))
                    for half, dre, dim_ in dests:
                        nc.vector.tensor_tensor(dre, tac[half], tbs[half], SUB)
                        nc.vector.tensor_tensor(dim_, tas[half], tbc[half], ADD)
                    if i == 2:
                        # transpose this quarter's v: vT [128, s] -> v [s, 128]
                        # via DMA xbar transpose (off the PE queue, ACT ring)
                        for t in range(4):
                            nc.sync.dma_start_transpose(
                                vsb[sq][:, t, :],
                                vTsb[sq][:, t * P:(t + 1) * P])

                if sq == 0:
                    # wo is needed only at phase 3; issue mid-phase-1 when the
                    # startup burst has drained
                    nc.scalar.dma_start(wo_sb[:], wo[:])

        # ---------- Phases 2+3: attention + output projection ----------
        # One pool context for both phases (closing a pool inserts an
        # all-engine barrier). Phase 2 runs as a single flat chunk stream
        # with a global 3-deep score->PV pipeline, so tile boundaries never
        # drain the ACT/DVE pipelines. Phase 3 reuses phase 2's PSUM tags.
        with tc.tile_pool(name="pt", bufs=7) as ptpool, \
             tc.tile_pool(name="st", bufs=4) as stpool, \
             tc.tile_pool(name="os", bufs=2) as ospool, \
             tc.tile_pool(name="ps2", bufs=1, space="PSUM") as pp2:

            pending_fin = [None]

            def emit_fin():
                if pending_fin[0] is not None:
                    fn, j, h = pending_fin[0]
                    pending_fin[0] = None
                    fn()
                    if h == NH_LOC - 1:
                        nc.gpsimd.collective_compute(
                            "AllGather", mybir.AluOpType.bypass,
                            replica_groups=[list(range(NCORES))],
                            ins=[cc_in[j][:]], outs=[cc_out[j][:]])

            class T2:
                __slots__ = ("j", "h", "nks", "ps_ctx", "ps_den")

                def __init__(self, j, h):
                    self.j, self.h, self.nks = j, h, 4 * (j + 1)
                    self.ps_ctx = self.ps_den = None

            def q_off(tl, t):
                return (t - 4 * tl.j) * P if t >= 4 * tl.j else 0

            def do_scores(tl, t):
                # Ragged diagonal: chunk t only attends queries >= its key
                # block, so compute just cols [q0:] and mask the one 128-col
                # triangle strip.
                j, h = tl.j, tl.h
                q0 = q_off(tl, t)
                ps_s = pp2.tile([P, SQ], F32, tag="s", bufs=3,
                                name=f"s{h}_{j}_{t}")
                nc.tensor.matmul(ps_s[:, q0:],
                                 kTsb[t // 4][:, (t % 4) * P:(t % 4 + 1) * P],
                                 qsb[j][:, h, q0:], start=True, stop=True)
                pT = ptpool.tile([P, SQ], BF16, tag="pT", name=f"pT{h}_{j}_{t}")
                # bias keeps exp small (cancels in softmax)
                nc.scalar.activation(pT[:, q0:], ps_s[:, q0:], EXP,
                                     scale=SCALE, bias=negb[:])
                if t >= 4 * j:
                    nc.vector.tensor_tensor(pT[:, q0:q0 + P], pT[:, q0:q0 + P],
                                            mask_sb[:, 0, 0:P], MULT)
                return pT

            def do_pv(tl, t, pT, first, last):
                if first:
                    # first processed chunk is always full-width, so every
                    # PSUM column is initialized by its start=True
                    tl.ps_ctx = pp2.tile([P, SQ], F32, tag="ctx", bufs=2,
                                         name=f"ctx{tl.h}_{tl.j}")
                    tl.ps_den = pp2.tile([1, SQ], F32, tag="den", bufs=2,
                                         name=f"den{tl.h}_{tl.j}")
                q0 = q_off(tl, t)
                nc.tensor.matmul(tl.ps_ctx[:, q0:], vsb[t // 4][:, t % 4, :],
                                 pT[:, q0:], start=first, stop=last)
                nc.tensor.matmul(tl.ps_den[0:1, q0:], ones_col[:], pT[:, q0:],
                                 start=first, stop=last)
                if last:
                    # 1/den via exp(-ln(den)) on ACT: Ln and Exp share one
                    # activation table set (no table switch with the attention
                    # exps), and the DVE's multi-pass reciprocal (3.4us per
                    # call) stays off the epilogue chain entirely.
                    lnden = stpool.tile([1, SQ], F32, tag="lnden")
                    nc.scalar.activation(lnden[:], tl.ps_den[0:1, :], LN)
                    rc_sb = stpool.tile([1, SQ], F32R, tag="rc_sb")
                    nc.scalar.activation(rc_sb[:], lnden[:], EXP, scale=-1.0)

                    def fin(tl=tl, rc_sb=rc_sb):
                        ps_bc = pp2.tile([P, SQ], F32, tag="bc", bufs=1,
                                         name=f"bc{tl.h}_{tl.j}")
                        nc.tensor.matmul(ps_bc[:], ones_row[:], rc_sb[:],
                                         start=True, stop=True)
                        bc_sb = stpool.tile([P, SQ], F32, tag="bc_sb")
                        nc.vector.tensor_copy(bc_sb[:], ps_bc[:])
                        ctx_st = stpool.tile([P, SQ], BF16, tag="ctx_st")
                        nc.vector.tensor_tensor(ctx_st[:], tl.ps_ctx[:],
                                                bc_sb[:], MULT)
                        nc.sync.dma_start(
                            cc_in[tl.j].rearrange("(h p) s -> p h s", p=P)
                                       [:, tl.h, :],
                            ctx_st[:])

                    pending_fin[0] = (fin, tl.j, tl.h)

            # flat stream over all (tile, chunk): diagonal chunks first
            # within each tile (tail flush then waits only on exp, not
            # exp+mask)
            stream = []
            for j in range(NSQ):
                for h in range(NH_LOC):
                    tl = T2(j, h)
                    order = list(range(4 * j, tl.nks)) + list(range(4 * j))
                    fin_t = 8 if tl.nks >= 12 else (6 if tl.nks >= 8 else 3)
                    for pos, t in enumerate(order):
                        stream.append((tl, t, pos, fin_t))

            from collections import deque
            pend = deque()
            for tl, t, pos, fin_t in stream:
                pend.append((tl, t, pos, do_scores(tl, t)))
                if pos == fin_t:
                    emit_fin()
                if len(pend) > 3:
                    ptl, pt, ppos, pT = pend.popleft()
                    do_pv(ptl, pt, pT, ppos == 0, ppos == ptl.nks - 1)
            while pend:
                ptl, pt, ppos, pT = pend.popleft()
                do_pv(ptl, pt, pT, ppos == 0, ppos == ptl.nks - 1)
            emit_fin()

            # ---- Phase 3: output projection (column shard) ----
            # cxt loads ride the ACT ring (the SP ring still has phase-2's
            # last cc_in writes queued); PSUM eviction goes to the idle
            # Vector engine. PSUM accumulators reuse phase-2 tags.
            for part in range(NSQ):
                ccv = cc_out[part].rearrange("(ko p) s -> p ko s", p=P)
                po = [pp2.tile([P, SQ], F32, tag="s", bufs=3,
                               name=f"o0_{part}"),
                      pp2.tile([P, SQ], F32, tag="s", bufs=3,
                               name=f"o1_{part}"),
                      pp2.tile([P, SQ], F32, tag="ctx", bufs=2,
                               name=f"o2_{part}"),
                      pp2.tile([P, SQ], F32, tag="ctx", bufs=2,
                               name=f"o3_{part}")]
                for ko in range(KO):
                    # cxt shares the pT pool: the slot-reuse dependency keeps
                    # the scheduler from hoisting these AG-gated loads ahead
                    # of phase-2's exps on the ACT ring (which would stall
                    # the exp stream behind a collective wait)
                    cxt = ptpool.tile([P, SQ], BF16, tag="pT",
                                      name=f"cx{part}_{ko}")
                    nc.scalar.dma_start(cxt[:], ccv[:, ko, :])
                    for dt in range(4):
                        nc.tensor.matmul(
                            po[dt][:],
                            wo_sb[:, ko, dt * P:(dt + 1) * P],
                            cxt[:],
                            start=(ko == 0), stop=(ko == KO - 1))
                out_sb = ospool.tile([P, 4, SQ], F32, tag="osb",
                                     name=f"osb{part}")
                for dt in range(4):
                    nc.vector.tensor_copy(out_sb[:, dt, :], po[dt][:])
                nc.sync.dma_start(
                    out.rearrange("(dt p) s -> p dt s", p=P)
                       [:, :, part * SQ:(part + 1) * SQ],
                    out_sb[:])

    nc.compile()
    _CACHE["nc"] = nc
    return nc


def _prep_inputs(x, wq, wk, wv, wo, freqs_cos, freqs_sin):
    """Host-side sharding + layout prep. Returns in_maps for the 8 cores."""
    x = np.asarray(x, dtype=np.float32)
    wq = np.asarray(wq, dtype=np.float32)
    wk = np.asarray(wk, dtype=np.float32)
    wv = np.asarray(wv, dtype=np.float32)
    wo = np.asarray(wo, dtype=np.float32)
    freqs_cos = np.asarray(freqs_cos, dtype=np.float32)
    freqs_sin = np.asarray(freqs_sin, dtype=np.float32)

    # xT in [P, NSQ, KO, SQ] layout: element (d, s), d = ko*128 + p, s = sq*SQ + s'
    xT = np.ascontiguousarray(
        x[0].T.reshape(KO, P, NSQ, SQ).transpose(1, 2, 0, 3)).astype(
            ml_dtypes.bfloat16)

    # rotate-half permutation within a head: [0,2,4,...126, 1,3,...,127]
    perm = np.concatenate([np.arange(0, HEAD_DIM, 2), np.arange(1, HEAD_DIM, 2)])

    # cos/sin tables transposed and duplicated across both 64-row halves
    cosT = np.ascontiguousarray(freqs_cos.T)  # [64, SEQ]
    sinT = np.ascontiguousarray(freqs_sin.T)
    cos2 = np.concatenate([cosT, cosT], axis=0)  # [128, SEQ]
    sin2 = np.concatenate([sinT, sinT], axis=0)

    # causal mask tiles: mask_r[i, jl] = 1 if jl - i >= 128*r
    i_idx = np.arange(P)[:, None]
    j_idx = np.arange(SQ)[None, :]
    masks = np.stack([(j_idx - i_idx >= P * r) for r in range(4)],
                     axis=0).astype(ml_dtypes.bfloat16)  # [4, 128, SQ]
    masks_l = np.ascontiguousarray(masks.transpose(1, 0, 2))  # [P, 4, SQ]

    in_maps = []
    for c in range(NCORES):
        # fused qkv weight rows, permuted for RoPE (re/im separated by m-tile)
        qh = [wq[(4 * c + h) * HEAD_DIM:(4 * c + h + 1) * HEAD_DIM][perm]
              for h in range(NH_LOC)]  # each [128, DIM], rows [re(64); im(64)]
        kh = wk[c * HEAD_DIM:(c + 1) * HEAD_DIM][perm]  # [128, DIM]
        vh = wv[c * HEAD_DIM:(c + 1) * HEAD_DIM]        # [128, DIM] original order
        rows = np.empty((MQKV, DIM), dtype=np.float32)
        rows[0:64] = qh[0][0:64]        # tile0: q0 re | q1 re
        rows[64:128] = qh[1][0:64]
        rows[128:192] = qh[2][0:64]     # tile1: q2 re | q3 re
        rows[192:256] = qh[3][0:64]
        rows[256:320] = kh[0:64]        # tile2: k re | v dims 0:64
        rows[320:384] = vh[0:64]
        rows[384:448] = qh[0][64:128]   # tile3: q0 im | q1 im
        rows[448:512] = qh[1][64:128]
        rows[512:576] = qh[2][64:128]   # tile4: q2 im | q3 im
        rows[576:640] = qh[3][64:128]
        rows[640:704] = kh[64:128]      # tile5: k im | v dims 64:128
        rows[704:768] = vh[64:128]
        wqkvT = np.ascontiguousarray(
            rows.T.reshape(KO, P, MQKV).transpose(1, 0, 2)).astype(
                ml_dtypes.bfloat16)

        woT = np.ascontiguousarray(
            wo[c * DOUT:(c + 1) * DOUT].T.reshape(KO, P, DOUT)
            .transpose(1, 0, 2)).astype(ml_dtypes.bfloat16)

        in_maps.append({
            "xt": xT,
            "wqkv": wqkvT,
            "wo": woT,
            "cost": cos2,
            "sint": sin2,
            "masks": masks_l,
        })
    return in_maps


def run(inputs, trace=False, tmpdir=None):
    """Compile (cached), run on 8 cores, return (output, BassKernelResults)."""
    from concourse.bass_utils import run_bass_kernel_spmd

    nc = _build()
    in_maps = _prep_inputs(**inputs)
    res = run_bass_kernel_spmd(nc, in_maps, list(range(NCORES)),
                               trace=trace, tmpdir=tmpdir)
    out = np.empty((BATCH, SEQ, DIM), dtype=np.float32)
    for c in range(NCORES):
        out[0, :, c * DOUT:(c + 1) * DOUT] = res.results[c]["o"].T
    return out, res


def kernel(**inputs) -> np.ndarray:
    out, _ = run(inputs)
    return out


# revision 31
# speedup vs baseline: 1.0661x; 1.0343x over previous
"""Tensor-parallel GQA attention kernel for 8 Trainium2 NeuronCores.

Sharding: head-parallel. Core c computes q heads [4c, 4c+4) and kv head c
(GQA group); the output projection is column-sharded after AllGathers of the
per-core context. The context AllGather is split into 4 seq quarters, each
triggered as soon as its q-tile's attention finishes, so the collectives
overlap attention and the output projection. Host concatenates the 8 output
shards.

Storage dtype is bf16 (fp16 matmuls run at half PE rate on TRN2 hardware;
bf16 runs at full rate and halves HBM traffic vs fp32). All matmul
accumulation is fp32 in PSUM; softmax denominators stay fp32.
"""

import math
import sys

import ml_dtypes
import numpy as np

sys.path.insert(0, "/opt/trn_rl_repo")

# ---- problem constants (hardcoded per harness contract) ----
DIM = 4096
N_HEADS = 32
N_KV_HEADS = 8
HEAD_DIM = 128
N_REP = 4
SEQ = 2048
BATCH = 1
NCORES = 8

P = 128
KO = DIM // P        # 32 contraction chunks
SQ = 512             # seq tile width (matmul moving free dim)
NSQ = SEQ // SQ      # 4
NKS = SEQ // P       # 16 key tiles of 128
NH_LOC = N_HEADS // NCORES   # 4 local q heads
MQKV = NH_LOC * HEAD_DIM + 2 * HEAD_DIM  # 768 rows of fused qkv projection
DOUT = DIM // NCORES  # 512 output columns per core
SCALE = 1.0 / math.sqrt(HEAD_DIM)

XB = 8               # k-chunks per xT load

_CACHE = {}


def _build():
    """Build and compile the Bass kernel once per process."""
    if "nc" in _CACHE:
        return _CACHE["nc"]

    import concourse.bacc as bacc
    import concourse.mybir as mybir
    import concourse.tile as tile
    from contextlib import ExitStack

    F32 = mybir.dt.float32
    F32R = mybir.dt.float32r
    BF16 = mybir.dt.bfloat16
    MULT = mybir.AluOpType.mult
    ADD = mybir.AluOpType.add
    SUB = mybir.AluOpType.subtract
    EXP = mybir.ActivationFunctionType.Exp
    LN = mybir.ActivationFunctionType.Ln

    nc = bacc.Bacc(None, target_bir_lowering=False, debug=False)

    xT = nc.declare_dram_parameter("xt", [P, NSQ, KO, SQ], BF16, isOutput=False)
    wqkv = nc.declare_dram_parameter("wqkv", [P, KO, MQKV], BF16, isOutput=False)
    wo = nc.declare_dram_parameter("wo", [P, KO, DOUT], BF16, isOutput=False)
    cosd = nc.declare_dram_parameter("cost", [P, SEQ], F32, isOutput=False)
    sind = nc.declare_dram_parameter("sint", [P, SEQ], F32, isOutput=False)
    maskd = nc.declare_dram_parameter("masks", [P, 4, SQ], BF16, isOutput=False)
    out = nc.declare_dram_parameter("o", [DOUT, SEQ], F32, isOutput=True)

    with tile.TileContext(nc) as tc, ExitStack() as stack, \
         nc.allow_low_precision(
             reason="bf16 storage; all matmul accumulation stays fp32 in PSUM"):
        singles = stack.enter_context(tc.tile_pool(name="singles", bufs=1))
        dram = stack.enter_context(tc.tile_pool(name="dram", bufs=1, space="DRAM"))

        # one AllGather per seq quarter, fired as each q-tile finishes
        cc_in = [dram.tile([NH_LOC * HEAD_DIM, SQ], BF16, name=f"ccin{j}")
                 for j in range(NSQ)]
        cc_out = [dram.tile([N_HEADS * HEAD_DIM, SQ], BF16, addr_space="Shared",
                            name=f"ccout{j}") for j in range(NSQ)]

        # small constants via vector engine (cheap, no DMA)
        ones_f = singles.tile([P, 1], F32)
        nc.vector.memset(ones_f[:], 1.0)
        ones_col = singles.tile([P, 1], BF16)
        nc.vector.tensor_copy(ones_col[:], ones_f[:])
        ones_all_f = singles.tile([P, P], F32)
        nc.vector.memset(ones_all_f[:], 1.0)
        ones_all = singles.tile([P, P], F32R)
        nc.vector.tensor_copy(ones_all[:], ones_all_f[:])
        negb = singles.tile([P, 1], F32)
        nc.vector.memset(negb[:], -5.0)

        # attention operands, resident across phases 1-2. Per-sq tiles so
        # phase-2 reads only depend on the sq quarter that produced them
        # (a single tile would serialize phase 2 behind ALL of phase 1).
        qsb = [singles.tile([P, NH_LOC, SQ], BF16, name=f"qsb{s}")
               for s in range(NSQ)]
        kTsb = [singles.tile([P, SQ], BF16, name=f"kt{s}") for s in range(NSQ)]
        vTsb = [singles.tile([P, SQ], BF16, name=f"vt{s}") for s in range(NSQ)]
        vsb = [singles.tile([P, 4, HEAD_DIM], BF16, name=f"v{s}")
               for s in range(NSQ)]

        cos_sb = singles.tile([P, SEQ], F32)
        sin_sb = singles.tile([P, SEQ], F32)
        mask_sb = singles.tile([P, 4, SQ], BF16)
        wo_sb = singles.tile([P, KO, DOUT], BF16)

        # ---------------- Phase 1: fused QKV projection + RoPE ----------------
        # m-tile order chosen so PSUM tiles are revisited in the order the
        # RoPE eviction frees them (pairs (0,3), (1,4), (2,5)).
        M_ORDER = (0, 3, 1, 4, 2, 5)
        M_ORDER_LAST = (2, 5, 0, 3, 1, 4)  # last k-chunk of last sq: stop k/v first
        with tc.tile_pool(name="wq", bufs=1) as wpool, \
             tc.tile_pool(name="xtp", bufs=3) as xpool, \
             tc.tile_pool(name="rt", bufs=2) as rpool, \
             tc.tile_pool(name="cp", bufs=2) as cpool, \
             tc.tile_pool(name="ps1", bufs=1, space="PSUM") as pp1:
            w = [None] * (KO // 4)

            def load_wg(g):
                wg = wpool.tile([P, 4, MQKV], BF16, tag=f"w{g}", name=f"w{g}")
                nc.sync.dma_start(wg[:], wqkv[:, 4 * g:4 * g + 4, :])
                w[g] = wg

            XCHUNKS = [(sq, xb) for sq in range(NSQ) for xb in range(KO // XB)]
            xtiles = {}

            def load_xk(i):
                sq, xb = XCHUNKS[i]
                xk = xpool.tile([P, XB, SQ], BF16, tag="xt", name=f"x{sq}_{xb}")
                nc.sync.dma_start(xk[:], xT[:, sq, xb * XB:(xb + 1) * XB, :])
                xtiles[i] = xk

            # startup order: weight groups and x tiles interleaved to match
            # the consumption order (PE eats one wg + one xk per 24 matmuls)
            # — the 8 cores' initial HBM burst is bandwidth-bound, so arrival
            # order is everything. masks are tiny; cos/sin are needed at the
            # first RoPE (~60us in); wo only at phase 3.
            load_wg(0)
            load_xk(0)
            nc.scalar.dma_start(mask_sb[:], maskd[:])
            load_wg(1)
            load_wg(2)
            load_xk(1)
            load_wg(3)
            for g in range(4, KO // 4):
                load_wg(g)
            nc.scalar.dma_start(cos_sb[:], cosd[:])
            nc.scalar.dma_start(sin_sb[:], sind[:])

            def wslice(k, m):
                return w[k // 4][:, k % 4, m * P:(m + 1) * P]

            for sq in range(NSQ):
                cols = slice(sq * SQ, (sq + 1) * SQ)
                pq = [pp1.tile([P, SQ], F32, tag=f"p{m}", name=f"p{m}_{sq}")
                      for m in range(6)]
                for xb in range(KO // XB):
                    i = sq * (KO // XB) + xb
                    if i + 2 < len(XCHUNKS):
                        load_xk(i + 2)
                    xk = xtiles.pop(i)
                    for kk in range(XB):
                        k = xb * XB + kk
                        morder = (M_ORDER_LAST if (sq == NSQ - 1 and k == KO - 1)
                                  else M_ORDER)
                        for m in morder:
                            nc.tensor.matmul(pq[m][:], wslice(k, m), xk[:, kk, :],
                                             start=(k == 0), stop=(k == KO - 1))

                # Evict PSUM -> SBUF fp32 on the (idle) Scalar engine first:
                # banks free ~0.5us per tile instead of waiting for the whole
                # DVE RoPE chain, so the next sq's matmuls (and phase 2's PSUM
                # pool, which needs every bank) never stall on the vector
                # engine. RoPE then runs from SBUF off the critical path.
                corder = M_ORDER_LAST if sq == NSQ - 1 else M_ORDER
                pcp = [None] * 6
                for m in corder:
                    pcp[m] = cpool.tile([P, SQ], F32, tag=f"c{m}",
                                        name=f"c{m}_{sq}")
                    nc.scalar.copy(pcp[m][:], pq[m][:])

                # RoPE. m-tile pairs: (0,3)->(q0,q1), (1,4)->(q2,q3),
                # (2,5)->(k | v-halves). Last sq handles k/v first so
                # attention + the v transposes start without waiting for the
                # q combines.
                groups = (2, 0, 1) if sq == NSQ - 1 else (0, 1, 2)
                for i in groups:
                    A, B = pcp[i][:], pcp[i + 3][:]
                    tac = rpool.tile([P, SQ], BF16, tag="tac")   # A*cos
                    tas = rpool.tile([P, SQ], BF16, tag="tas")   # A*sin
                    tbs = rpool.tile([P, SQ], BF16, tag="tbs")   # B*sin
                    tbc = rpool.tile([P, SQ], BF16, tag="tbc")   # B*cos
                    nc.vector.tensor_tensor(tac[:], A, cos_sb[:, cols], MULT)
                    nc.vector.tensor_tensor(tas[:], A, sin_sb[:, cols], MULT)
                    if i == 2:
                        # v passthrough straight from PSUM (frees pq[2]/pq[5])
                        nc.vector.tensor_copy(vTsb[sq][0:64, :], A[64:128])
                    nc.vector.tensor_tensor(tbs[:], B, sin_sb[:, cols], MULT)
                    nc.vector.tensor_tensor(tbc[:], B, cos_sb[:, cols], MULT)
                    if i == 2:
                        nc.vector.tensor_copy(vTsb[sq][64:128, :], B[64:128])
                        dests = ((slice(0, 64), kTsb[sq][0:64, :],
                                  kTsb[sq][64:128, :]),)
                    else:
                        h0q, h1q = 2 * i, 2 * i + 1
                        dests = ((slice(0, 64), qsb[sq][0:64, h0q, :],
                                  qsb[sq][64:128, h0q, :]),
                                 (slice(64, 128), qsb[sq][0:64, h1q, :],
                                  qsb[sq][64:128, h1q, :]))
                    for half, dre, dim_ in dests:
                        nc.vector.tensor_tensor(dre, tac[half], tbs[half], SUB)
                        nc.vector.tensor_tensor(dim_, tas[half], tbc[half], ADD)
                    if i == 2:
                        # transpose this quarter's v: vT [128, s] -> v [s, 128]
                        # via DMA xbar transpose (off the PE queue, ACT ring)
                        for t in range(4):
                            nc.sync.dma_start_transpose(
                                vsb[sq][:, t, :],
                                vTsb[sq][:, t * P:(t + 1) * P])

                if sq == 0:
                    # wo is needed only at phase 3; issue mid-phase-1 when the
                    # startup burst has drained
                    nc.scalar.dma_start(wo_sb[:], wo[:])

        # ---------- Phases 2+3: attention + output projection ----------
        # One pool context for both phases (closing a pool inserts an
        # all-engine barrier). Phase 2 runs as a single flat chunk stream
        # with a global 3-deep score->PV pipeline, so tile boundaries never
        # drain the ACT/DVE pipelines. Phase 3 reuses phase 2's PSUM tags.
        with tc.tile_pool(name="pt", bufs=7) as ptpool, \
             tc.tile_pool(name="st", bufs=4) as stpool, \
             tc.tile_pool(name="os", bufs=2) as ospool, \
             tc.tile_pool(name="ps2", bufs=1, space="PSUM") as pp2:

            pending_fins = []
            finished_j = [None]

            def emit_fin():
                if pending_fins:
                    fn, j, h = pending_fins.pop(0)
                    fn()
                    if h == NH_LOC - 1:
                        nc.gpsimd.collective_compute(
                            "AllGather", mybir.AluOpType.bypass,
                            replica_groups=[list(range(NCORES))],
                            ins=[cc_in[j][:]], outs=[cc_out[j][:]])

            class T2:
                __slots__ = ("j", "h", "nks", "ps_ctx", "ps_den", "jd", "ctx_un")

                def __init__(self, j, h, jd):
                    self.j, self.h, self.nks = j, h, 4 * (j + 1)
                    self.jd = jd
                    self.ps_ctx = self.ps_den = self.ctx_un = None

            class JDen:
                """Per-j [4, SQ] denominator gather (one row per head)."""
                __slots__ = ("j", "den4", "tiles")

                def __init__(self, j):
                    self.j = j
                    self.den4 = None
                    self.tiles = []

            def q_off(tl, t):
                return (t - 4 * tl.j) * P if t >= 4 * tl.j else 0

            def do_scores(tl, t):
                # Ragged diagonal: chunk t only attends queries >= its key
                # block, so compute just cols [q0:] and mask the one 128-col
                # triangle strip.
                j, h = tl.j, tl.h
                q0 = q_off(tl, t)
                ps_s = pp2.tile([P, SQ], F32, tag="s", bufs=3,
                                name=f"s{h}_{j}_{t}")
                nc.tensor.matmul(ps_s[:, q0:],
                                 kTsb[t // 4][:, (t % 4) * P:(t % 4 + 1) * P],
                                 qsb[j][:, h, q0:], start=True, stop=True)
                pT = ptpool.tile([P, SQ], BF16, tag="pT", name=f"pT{h}_{j}_{t}")
                # bias keeps exp small (cancels in softmax)
                nc.scalar.activation(pT[:, q0:], ps_s[:, q0:], EXP,
                                     scale=SCALE, bias=negb[:])
                if t >= 4 * j:
                    nc.vector.tensor_tensor(pT[:, q0:q0 + P], pT[:, q0:q0 + P],
                                            mask_sb[:, 0, 0:P], MULT)
                return pT

            def do_pv(tl, t, pT, first, last):
                h = tl.h
                if first:
                    # first processed chunk is always full-width, so every
                    # PSUM column is initialized by its start=True
                    tl.ps_ctx = pp2.tile([P, SQ], F32, tag="ctx", bufs=2,
                                         name=f"ctx{h}_{tl.j}")
                    tl.ps_den = pp2.tile([1, SQ], F32, tag="den", bufs=2,
                                         name=f"den{h}_{tl.j}")
                    if h == 0:
                        # head rows at partitions 0/32/64/96 (engine accesses
                        # must start on a 32-aligned partition)
                        tl.jd.den4 = stpool.tile([P, SQ], F32, tag="d4",
                                                 name=f"d4_{tl.j}")
                q0 = q_off(tl, t)
                nc.tensor.matmul(tl.ps_ctx[:, q0:], vsb[t // 4][:, t % 4, :],
                                 pT[:, q0:], start=first, stop=last)
                nc.tensor.matmul(tl.ps_den[0:1, q0:], ones_col[:],
                                 pT[:, q0:], start=first, stop=last)
                if last:
                    # evict ctx unnormalized (bf16) and gather this head's
                    # denominator row — frees both PSUM banks at tile end
                    # instead of holding them through the epilogue
                    tl.ctx_un = stpool.tile([P, SQ], BF16, tag="cun",
                                            name=f"cun{h}_{tl.j}")
                    nc.vector.tensor_copy(tl.ctx_un[:], tl.ps_ctx[:])
                    nc.vector.tensor_copy(tl.jd.den4[32 * h:32 * h + 1, :],
                                          tl.ps_den[0:1, :])
                    tl.jd.tiles.append(tl)
                    if h == NH_LOC - 1:
                        finish_j(tl.jd)

            def finish_j(jd):
                # One batched reciprocal per j: the DVE's multi-pass
                # reciprocal costs ~3.2us per CALL (free-dim length), so
                # [4,SQ] covers all 4 heads at once; no ACT table switches.
                # full-width reciprocal costs the same as 4 rows (the DVE's
                # multi-pass cost is free-dim bound); unused rows are junk
                rc4 = stpool.tile([P, SQ], F32R, tag="rc4", name=f"rc4_{jd.j}")
                nc.vector.reciprocal(rc4[:], jd.den4[:])
                # matmul moving operands must start at partition 0/32/64:
                # only head 3 (row 96) needs relocation
                rcs2 = stpool.tile([1, SQ], F32R, tag="rcs2",
                                   name=f"rcs2_{jd.j}")
                nc.vector.tensor_copy(rcs2[:], rc4[96:97, :])
                for tl in jd.tiles:
                    def fin(tl=tl, rc4=rc4, rcs2=rcs2):
                        h = tl.h
                        stat = (ones_all[32 * h:32 * h + 1, :] if h < 3
                                else ones_all[0:1, :])
                        mov = (rc4[32 * h:32 * h + 1, :] if h < 3
                               else rcs2[0:1, :])
                        ps_bc = pp2.tile([P, SQ], F32, tag="bc", bufs=1,
                                         name=f"bc{tl.h}_{tl.j}")
                        nc.tensor.matmul(ps_bc[:], stat, mov,
                                         start=True, stop=True)
                        ctx_st = stpool.tile([P, SQ], BF16, tag="ctx_st")
                        nc.vector.tensor_tensor(ctx_st[:], tl.ctx_un[:],
                                                ps_bc[:], MULT)
                        nc.sync.dma_start(
                            cc_in[tl.j].rearrange("(h p) s -> p h s", p=P)
                                       [:, tl.h, :],
                            ctx_st[:])

                    pending_fins.append((fin, tl.j, tl.h))

            # flat stream over all (tile, chunk): diagonal chunks first
            # within each tile (tail flush then waits only on exp, not
            # exp+mask)
            stream = []
            for j in range(NSQ):
                jd = JDen(j)
                for h in range(NH_LOC):
                    tl = T2(j, h, jd)
                    order = list(range(4 * j, tl.nks)) + list(range(4 * j))
                    for pos, t in enumerate(order):
                        stream.append((tl, t, pos))

            from collections import deque
            pend = deque()
            for tl, t, pos in stream:
                pend.append((tl, t, pos, do_scores(tl, t)))
                if pos >= 6 and pending_fins:
                    emit_fin()
                if len(pend) > 3:
                    ptl, pt, ppos, pT = pend.popleft()
                    do_pv(ptl, pt, pT, ppos == 0, ppos == ptl.nks - 1)
            while pend:
                ptl, pt, ppos, pT = pend.popleft()
                do_pv(ptl, pt, pT, ppos == 0, ppos == ptl.nks - 1)
            while pending_fins:
                emit_fin()

            # ---- Phase 3: output projection (column shard) ----
            # cxt loads ride the ACT ring (the SP ring still has phase-2's
            # last cc_in writes queued); PSUM eviction goes to the idle
            # Vector engine. PSUM accumulators reuse phase-2 tags.
            for part in range(NSQ):
                ccv = cc_out[part].rearrange("(ko p) s -> p ko s", p=P)
                po = [pp2.tile([P, SQ], F32, tag="s", bufs=3,
                               name=f"o0_{part}"),
                      pp2.tile([P, SQ], F32, tag="s", bufs=3,
                               name=f"o1_{part}"),
                      pp2.tile([P, SQ], F32, tag="ctx", bufs=2,
                               name=f"o2_{part}"),
                      pp2.tile([P, SQ], F32, tag="ctx", bufs=2,
                               name=f"o3_{part}")]
                for ko in range(KO):
                    # cxt shares the pT pool: the slot-reuse dependency keeps
                    # the scheduler from hoisting these AG-gated loads ahead
                    # of phase-2's exps on the ACT ring (which would stall
                    # the exp stream behind a collective wait)
                    cxt = ptpool.tile([P, SQ], BF16, tag="pT",
                                      name=f"cx{part}_{ko}")
                    nc.scalar.dma_start(cxt[:], ccv[:, ko, :])
                    for dt in range(4):
                        nc.tensor.matmul(
                            po[dt][:],
                            wo_sb[:, ko, dt * P:(dt + 1) * P],
                            cxt[:],
                            start=(ko == 0), stop=(ko == KO - 1))
                out_sb = ospool.tile([P, 4, SQ], F32, tag="osb",
                                     name=f"osb{part}")
                for dt in range(4):
                    nc.vector.tensor_copy(out_sb[:, dt, :], po[dt][:])
                nc.sync.dma_start(
                    out.rearrange("(dt p) s -> p dt s", p=P)
                       [:, :, part * SQ:(part + 1) * SQ],
                    out_sb[:])

    nc.compile()
    _CACHE["nc"] = nc
    return nc


def _prep_inputs(x, wq, wk, wv, wo, freqs_cos, freqs_sin):
    """Host-side sharding + layout prep. Returns in_maps for the 8 cores."""
    x = np.asarray(x, dtype=np.float32)
    wq = np.asarray(wq, dtype=np.float32)
    wk = np.asarray(wk, dtype=np.float32)
    wv = np.asarray(wv, dtype=np.float32)
    wo = np.asarray(wo, dtype=np.float32)
    freqs_cos = np.asarray(freqs_cos, dtype=np.float32)
    freqs_sin = np.asarray(freqs_sin, dtype=np.float32)

    # xT in [P, NSQ, KO, SQ] layout: element (d, s), d = ko*128 + p, s = sq*SQ + s'
    xT = np.ascontiguousarray(
        x[0].T.reshape(KO, P, NSQ, SQ).transpose(1, 2, 0, 3)).astype(
            ml_dtypes.bfloat16)

    # rotate-half permutation within a head: [0,2,4,...126, 1,3,...,127]
    perm = np.concatenate([np.arange(0, HEAD_DIM, 2), np.arange(1, HEAD_DIM, 2)])

    # cos/sin tables transposed and duplicated across both 64-row halves
    cosT = np.ascontiguousarray(freqs_cos.T)  # [64, SEQ]
    sinT = np.ascontiguousarray(freqs_sin.T)
    cos2 = np.concatenate([cosT, cosT], axis=0)  # [128, SEQ]
    sin2 = np.concatenate([sinT, sinT], axis=0)

    # causal mask tiles: mask_r[i, jl] = 1 if jl - i >= 128*r
    i_idx = np.arange(P)[:, None]
    j_idx = np.arange(SQ)[None, :]
    masks = np.stack([(j_idx - i_idx >= P * r) for r in range(4)],
                     axis=0).astype(ml_dtypes.bfloat16)  # [4, 128, SQ]
    masks_l = np.ascontiguousarray(masks.transpose(1, 0, 2))  # [P, 4, SQ]

    in_maps = []
    for c in range(NCORES):
        # fused qkv weight rows, permuted for RoPE (re/im separated by m-tile)
        qh = [wq[(4 * c + h) * HEAD_DIM:(4 * c + h + 1) * HEAD_DIM][perm]
              for h in range(NH_LOC)]  # each [128, DIM], rows [re(64); im(64)]
        kh = wk[c * HEAD_DIM:(c + 1) * HEAD_DIM][perm]  # [128, DIM]
        vh = wv[c * HEAD_DIM:(c + 1) * HEAD_DIM]        # [128, DIM] original order
        rows = np.empty((MQKV, DIM), dtype=np.float32)
        rows[0:64] = qh[0][0:64]        # tile0: q0 re | q1 re
        rows[64:128] = qh[1][0:64]
        rows[128:192] = qh[2][0:64]     # tile1: q2 re | q3 re
        rows[192:256] = qh[3][0:64]
        rows[256:320] = kh[0:64]        # tile2: k re | v dims 0:64
        rows[320:384] = vh[0:64]
        rows[384:448] = qh[0][64:128]   # tile3: q0 im | q1 im
        rows[448:512] = qh[1][64:128]
        rows[512:576] = qh[2][64:128]   # tile4: q2 im | q3 im
        rows[576:640] = qh[3][64:128]
        rows[640:704] = kh[64:128]      # tile5: k im | v dims 64:128
        rows[704:768] = vh[64:128]
        wqkvT = np.ascontiguousarray(
            rows.T.reshape(KO, P, MQKV).transpose(1, 0, 2)).astype(
                ml_dtypes.bfloat16)

        woT = np.ascontiguousarray(
            wo[c * DOUT:(c + 1) * DOUT].T.reshape(KO, P, DOUT)
            .transpose(1, 0, 2)).astype(ml_dtypes.bfloat16)

        in_maps.append({
            "xt": xT,
            "wqkv": wqkvT,
            "wo": woT,
            "cost": cos2,
            "sint": sin2,
            "masks": masks_l,
        })
    return in_maps


def run(inputs, trace=False, tmpdir=None):
    """Compile (cached), run on 8 cores, return (output, BassKernelResults)."""
    from concourse.bass_utils import run_bass_kernel_spmd

    nc = _build()
    in_maps = _prep_inputs(**inputs)
    res = run_bass_kernel_spmd(nc, in_maps, list(range(NCORES)),
                               trace=trace, tmpdir=tmpdir)
    out = np.empty((BATCH, SEQ, DIM), dtype=np.float32)
    for c in range(NCORES):
        out[0, :, c * DOUT:(c + 1) * DOUT] = res.results[c]["o"].T
    return out, res


def kernel(**inputs) -> np.ndarray:
    out, _ = run(inputs)
    return out
